# revision 15
# baseline (speedup 1.0000x reference)
"""Trainium2 Bass kernel for: conv3x3 -> conv3x3 -> maxpool2x2 -> conv3x3 -> conv3x3
on a [1,1,8192,8192] fp32 image, SAME padding, single channel.

Strategy (8 NeuronCores, height-sharded, halo replicated on host — no collectives):
  * conv1*conv2 are composed into one 5x5 correlation ("stage A"); likewise
    conv3*conv4 ("stage B"). Each 5x5 is computed as 5 PSUM-accumulated
    TensorE band matmuls: the stationary [K<=128, M<=124] band matrix carries
    the 5 vertical taps (mapping input rows on partitions -> output rows),
    and the 5 horizontal taps come from shifting the moving operand's column
    window by t=0..4.
  * Fusing two SAME convs is NOT a plain 5x5 at the image border (the
    reference zeroes the intermediate ring). All corrections are folded into
    the band-matrix *data*: edge-row edits in the main bands, plus per-block
    single-column correction matmuls (bandL/bandR) for the left/right image
    columns, with corner fix-ups. Per-core variants also zero the phantom
    pooled halo rows. The SPMD program is identical on all 8 cores; only the
    band-matrix values differ per core.
  * maxpool2x2: stage-A bands write even/odd output rows to separate
    partition groups, so the row-pair max is a plain partition-sliced
    tensor_tensor max; the column-pair max uses stride-2 access patterns.
    Pooled rows are assembled directly into SBUF-resident stage-B input
    tiles; stage B never touches HBM for its input.
"""

import numpy as np

try:
    import concourse.bass  # noqa: F401
except ImportError:
    import sys
    sys.path.insert(0, "/opt/trn_rl_repo")

H = 8192
W = 8192
NCORES = 8
RPC = H // NCORES          # x rows per core
OPC = RPC // 2             # output rows per core
NZ = RPC + 8               # stage-A output rows computed per core
BM = 124                   # output rows per band-matmul block
NBLK = (NZ + BM - 1) // BM           # 9 stage-A blocks
NBB = (OPC + BM - 1) // BM           # 5 stage-B blocks
WH = W // 2
NSTRIPE = 2
SW = W // NSTRIPE          # output cols per stage-A stripe
CH = 512                   # psum chunk width
NCHUNK_A = SW // CH        # 8
NCHUNK_B = WH // CH        # 8
XROWS = RPC + 16
XCOLS = W + 4
DT_F32 = None  # filled lazily (mybir.dt.float32)


# ------------------------------------------------------------------ bands ---

def _conv_full2d(a, b):
    na, ma = a.shape
    nb, mb = b.shape
    out = np.zeros((na + nb - 1, ma + mb - 1), dtype=np.float64)
    for i in range(na):
        for j in range(ma):
            out[i : i + nb, j : j + mb] += a[i, j] * b
    return out


def _stage_consts(w1, w2):
    w1 = np.asarray(w1, np.float64)
    w2 = np.asarray(w2, np.float64)
    return dict(
        K5=_conv_full2d(w1, w2),
        kh0=np.convolve(w2[0, :], w1[2, :]),
        khb=np.convolve(w2[2, :], w1[0, :]),
        kv0=np.convolve(w2[:, 0], w1[:, 2]),
        kvW=np.convolve(w2[:, 2], w1[:, 0]),
        c00=w2[0, 0] * w1[2, 2],
        c0W=w2[0, 2] * w1[2, 0],
        cH0=w2[2, 0] * w1[0, 2],
        cHW=w2[2, 2] * w1[0, 0],
    )


def _rowmap_permuted(M):
    h = M // 2
    return np.array([2 * m if m < h else 2 * (m - h) + 1 for m in range(M)])


def _build_stage_bands(C, K, M, rowmap, glob_rows, Hout, zero_rows=()):
    """bands [5][K, M], bandL [K, M], bandR [K, M] (float64)."""
    bands = np.zeros((5, K, M), dtype=np.float64)
    bandL = np.zeros((K, M), dtype=np.float64)
    bandR = np.zeros((K, M), dtype=np.float64)
    for m in range(M):
        r = rowmap[m]
        for a in range(5):
            k = r + a
            if k >= K:
                continue
            bands[:, k, m] = C["K5"][a, :]
            bandL[k, m] = -C["kv0"][a]
            bandR[k, m] = -C["kvW"][a]
        g = glob_rows[m]
        k2 = r + 2
        if k2 < K:
            if g == 0:
                bands[:, k2, m] -= C["kh0"]
                bandL[k2, m] += C["c00"]
                bandR[k2, m] += C["c0W"]
            if g == Hout - 1:
                bands[:, k2, m] -= C["khb"]
                bandL[k2, m] += C["cH0"]
                bandR[k2, m] += C["cHW"]
    for k in zero_rows:
        bands[:, k, :] = 0.0
        bandL[k, :] = 0.0
        bandR[k, :] = 0.0
    return bands, bandL, bandR


def _pack(bands):
    """[5, K, M] -> [K, 5*M] matching lhsT slices [K, t*M:(t+1)*M]."""
    b, K, M = bands.shape
    return np.ascontiguousarray(
        bands.transpose(1, 0, 2).reshape(K, b * M).astype(np.float32)
    )


def _scatter_m128(bands, bandL, bandR, Mz):
    """Place dense output cols into the 128-wide layout: even rows at
    partitions 0..h-1, odd rows at 64..64+h-1 (rest zero)."""
    h = Mz // 2
    K = bands.shape[1]
    b128 = np.zeros((5, K, 128), dtype=np.float64)
    l128 = np.zeros((K, 128), dtype=np.float64)
    r128 = np.zeros((K, 128), dtype=np.float64)
    for m in range(Mz):
        col = m if m < h else 64 + (m - h)
        b128[:, :, col] = bands[:, :, m]
        l128[:, col] = bandL[:, m]
        r128[:, col] = bandR[:, m]
    return b128, l128, r128


def _core_weights(c, w1, w2, w3, w4):
    """All weight arrays for core c, keyed by DRAM tensor name."""
    CA = _stage_consts(w1, w2)
    CB = _stage_consts(w3, w4)
    out = {}
    far = np.full(BM, 10 ** 9)  # glob rows that trigger no edits

    # --- stage A ---
    def bands_a(blk):
        Mz = min(BM, NZ - BM * blk)
        Kx = Mz + 4
        rm = _rowmap_permuted(Mz)
        glob = np.array([RPC * c - 4 + BM * blk + rm[m] for m in range(Mz)])
        return _scatter_m128(*_build_stage_bands(CA, Kx, Mz, rm, glob, H), Mz)

    gen = _scatter_m128(
        *_build_stage_bands(CA, 128, BM, _rowmap_permuted(BM), far, H), BM
    )
    blk0 = bands_a(0)
    blk8 = bands_a(NBLK - 1)
    out["wa"], out["wla"], out["wra"] = _pack(gen[0]), *(
        np.ascontiguousarray(g.astype(np.float32)) for g in gen[1:]
    )
    out["wa0"], out["wla0"], out["wra0"] = _pack(blk0[0]), *(
        np.ascontiguousarray(g.astype(np.float32)) for g in blk0[1:]
    )
    out["wa8"], out["wla8"], out["wra8"] = _pack(blk8[0]), *(
        np.ascontiguousarray(g.astype(np.float32)) for g in blk8[1:]
    )

    # --- stage B ---
    def bands_b(d):
        Mo = min(BM, OPC - BM * d)
        Kp = Mo + 4
        rm = np.arange(Mo)
        glob = np.array([OPC * c + BM * d + m for m in range(Mo)])
        zero = [
            k
            for k in range(Kp)
            if not (0 <= OPC * c - 2 + BM * d + k < H // 2)
        ]
        return _build_stage_bands(CB, Kp, Mo, rm, glob, H // 2, zero)

    genb = _build_stage_bands(CB, 128, BM, np.arange(BM), far, H // 2)
    d0 = bands_b(0)
    d4 = bands_b(NBB - 1)
    out["wb"], out["wlb"], out["wrb"] = _pack(genb[0]), *(
        np.ascontiguousarray(g.astype(np.float32)) for g in genb[1:]
    )
    out["wb0"], out["wlb0"], out["wrb0"] = _pack(d0[0]), *(
        np.ascontiguousarray(g.astype(np.float32)) for g in d0[1:]
    )
    out["wb4"], out["wlb4"], out["wrb4"] = _pack(d4[0]), *(
        np.ascontiguousarray(g.astype(np.float32)) for g in d4[1:]
    )
    return out


def _local_x(x2, c):
    """[XROWS, XCOLS] zero-padded slice for core c (rows RPC*c-6 .., cols -2..)."""
    r0 = RPC * c - 6
    out = np.zeros((XROWS, XCOLS), dtype=np.float32)
    rlo, rhi = max(r0, 0), min(r0 + XROWS, H)
    out[rlo - r0 : rhi - r0, 2 : 2 + W] = x2[rlo:rhi]
    return out


# ---------------------------------------------------------------- program ---

_PROGRAM = {}


MM_DTYPE = "float32r"  # "float32" (4 cyc/row, exact) or "float32r" (1 cyc/row)


def _build_program():
    import concourse.bacc as bacc
    import concourse.mybir as mybir
    import concourse.tile as tile

    f32 = mybir.dt.float32
    mmdt = getattr(mybir.dt, MM_DTYPE)

    def mm(ap):
        return ap

    nc = bacc.Bacc("TRN2")

    xdram = nc.dram_tensor("x", [XROWS, XCOLS], mmdt, kind="ExternalInput")
    wshape = {
        "wa": [128, 5 * 128], "wa0": [128, 5 * 128], "wa8": [44, 5 * 128],
        "wla": [128, 128], "wra": [128, 128],
        "wla0": [128, 128], "wra0": [128, 128],
        "wla8": [44, 128], "wra8": [44, 128],
        "wb": [128, 5 * BM], "wb0": [128, 5 * BM], "wb4": [20, 5 * 16],
        "wlb": [128, BM], "wrb": [128, BM],
        "wlb0": [128, BM], "wrb0": [128, BM],
        "wlb4": [20, 16], "wrb4": [20, 16],
    }
    wdram = {
        k: nc.dram_tensor(k, v, mmdt, kind="ExternalInput")
        for k, v in wshape.items()
    }
    zdram = nc.dram_tensor("zpad", [128, 4], mmdt, kind="ExternalInput")
    outdram = nc.dram_tensor("out", [OPC, WH], f32, kind="ExternalOutput")

    with tile.TileContext(nc) as tc:
        with (
            tc.tile_pool(name="wpool", bufs=1) as wpool,
            tc.tile_pool(name="xpool", bufs=2) as xpool,
            tc.tile_pool(name="zpool", bufs=3) as zpool,
            tc.tile_pool(name="hpool", bufs=3) as hpool,
            tc.tile_pool(name="h2pool", bufs=3) as h2pool,
            tc.tile_pool(name="spool", bufs=2) as spool,
            tc.tile_pool(name="ppool", bufs=1) as ppool,
            tc.tile_pool(name="opool", bufs=2) as opool,
            tc.tile_pool(name="zps_pool", bufs=4, space="PSUM") as zps_pool,
            tc.tile_pool(name="bps_pool", bufs=2, space="PSUM") as bps_pool,
            tc.tile_pool(name="cps_pool", bufs=2, space="PSUM") as cps_pool,
        ):
            wt = {}
            for k, shp in wshape.items():
                wt[k] = wpool.tile(shp, mmdt, name=f"wt_{k}")
                nc.sync.dma_start(out=wt[k], in_=wdram[k][:])

            pooled = []
            for d in range(NBB):
                pt = ppool.tile([128, WH + 4], mmdt, name=f"pooled{d}")
                pooled.append(pt)
                nc.sync.dma_start(out=pt[:, 0:2], in_=zdram[:, 0:2])
                nc.sync.dma_start(out=pt[:, WH + 2 : WH + 4], in_=zdram[:, 2:4])

            # ---------------- stage A ----------------
            for b in range(NBLK):
                Mz = min(BM, NZ - BM * b)
                h = Mz // 2
                Kx = Mz + 4
                wa_t = wt["wa0"] if b == 0 else (wt["wa8"] if b == NBLK - 1 else wt["wa"])
                wl_t = wt["wla0"] if b == 0 else (wt["wla8"] if b == NBLK - 1 else wt["wla"])
                wr_t = wt["wra0"] if b == 0 else (wt["wra8"] if b == NBLK - 1 else wt["wra"])
                d_t, po = divmod(62 * b, BM)
                for s in range(NSTRIPE):
                    st = spool.tile([64, SW // 2 + 4], mmdt, name="stg")
                    xt = xpool.tile([128, SW + 4], mmdt, name="xt")
                    nc.sync.dma_start(
                        out=xt[0:Kx],
                        in_=xdram[BM * b : BM * b + Kx, SW * s : SW * s + SW + 4],
                    )
                    for jj in range(NCHUNK_A):
                        zps = zps_pool.tile([128, CH], f32, name="zps")
                        corr = ("L" if (s == 0 and jj == 0) else
                                "R" if (s == NSTRIPE - 1 and jj == NCHUNK_A - 1) else None)
                        for t in range(5):
                            nc.tensor.matmul(
                                zps,
                                lhsT=mm(wa_t[0:Kx, t * 128 : (t + 1) * 128]),
                                rhs=mm(xt[0:Kx, CH * jj + t : CH * jj + t + CH]),
                                start=(t == 0),
                                stop=(t == 4),
                            )
                        cps = None
                        if corr == "L":
                            # rhs col0 = padded zero, col1 = x col 0
                            cps = cps_pool.tile([128, 2], f32, name="cps", tag="cps")
                            nc.tensor.matmul(
                                cps, lhsT=mm(wl_t[0:Kx]),
                                rhs=mm(xt[0:Kx, 1:3]), start=True, stop=True,
                            )
                        elif corr == "R":
                            # rhs col0 = x col W-1, col1 = padded zero
                            cps = cps_pool.tile([128, 2], f32, name="cps", tag="cps")
                            nc.tensor.matmul(
                                cps, lhsT=mm(wr_t[0:Kx]),
                                rhs=mm(xt[0:Kx, SW + 1 : SW + 3]), start=True, stop=True,
                            )
                        zsb = zpool.tile([128, CH], f32, name="zsb")
                        nc.scalar.copy(out=zsb, in_=zps)
                        if corr == "L":
                            nc.vector.tensor_add(
                                out=zsb[:, 0:1], in0=zsb[:, 0:1], in1=cps[:, 1:2]
                            )
                        elif corr == "R":
                            nc.vector.tensor_add(
                                out=zsb[:, CH - 1 : CH], in0=zsb[:, CH - 1 : CH],
                                in1=cps[:, 0:1],
                            )
                        hp = hpool.tile([128, CH // 2], f32, name="hp")
                        nc.vector.tensor_max(
                            out=hp, in0=zsb[:, 0:CH:2], in1=zsb[:, 1:CH:2]
                        )
                        # row-pair max: TT needs equal base partitions, so
                        # first move the odd-row half down to base 0
                        hp2 = h2pool.tile([64, CH // 2], f32, name="hp2")
                        nc.vector.tensor_copy(out=hp2, in_=hp[64:128])
                        col0 = 2 + (CH // 2) * jj
                        nc.vector.tensor_max(
                            out=st[:, col0 : col0 + CH // 2],
                            in0=hp[0:64],
                            in1=hp2,
                        )
                    # scatter the stripe's pooled rows into the stage-B input
                    # tiles (DMA: compute engines can't address partition 62)
                    pc0 = 2 + (SW // 2) * s
                    nc.sync.dma_start(
                        out=pooled[d_t][po : po + h, pc0 : pc0 + SW // 2],
                        in_=st[0:h, 2 : SW // 2 + 2],
                    )
                    if b >= 2 and b % 2 == 0:
                        # pooled tiles overlap by 4 rows at band boundaries
                        nc.sync.dma_start(
                            out=pooled[b // 2 - 1][BM : BM + 4, pc0 : pc0 + SW // 2],
                            in_=st[0:4, 2 : SW // 2 + 2],
                        )

            # ---------------- stage B ----------------
            for d in range(NBB):
                Mo = min(BM, OPC - BM * d)
                Kp = Mo + 4
                wb_t = wt["wb0"] if d == 0 else (wt["wb4"] if d == NBB - 1 else wt["wb"])
                wlb_t = wt["wlb0"] if d == 0 else (wt["wlb4"] if d == NBB - 1 else wt["wlb"])
                wrb_t = wt["wrb0"] if d == 0 else (wt["wrb4"] if d == NBB - 1 else wt["wrb"])
                for half in range(2):
                    osb = opool.tile([BM, WH // 2], f32, name="osb")
                    for jh in range(NCHUNK_B // 2):
                        jj = half * (NCHUNK_B // 2) + jh
                        bps = bps_pool.tile([BM, CH], f32, name="bps")
                        corr = ("L" if jj == 0 else
                                "R" if jj == NCHUNK_B - 1 else None)
                        for t in range(5):
                            nc.tensor.matmul(
                                bps[0:Mo],
                                lhsT=mm(wb_t[0:Kp, t * Mo : (t + 1) * Mo]),
                                rhs=mm(pooled[d][0:Kp, CH * jj + t : CH * jj + t + CH]),
                                start=(t == 0),
                                stop=(t == 4),
                            )
                        cps = None
                        if corr == "L":
                            # rhs col0 = zero pad col, col1 = pooled col 0
                            cps = cps_pool.tile([128, 2], f32, name="cpsb", tag="cps")
                            nc.tensor.matmul(
                                cps[0:Mo], lhsT=mm(wlb_t[0:Kp, 0:Mo]),
                                rhs=mm(pooled[d][0:Kp, 1:3]), start=True, stop=True,
                            )
                        elif corr == "R":
                            # rhs col0 = pooled col WH-1, col1 = zero pad col
                            cps = cps_pool.tile([128, 2], f32, name="cpsb", tag="cps")
                            nc.tensor.matmul(
                                cps[0:Mo], lhsT=mm(wrb_t[0:Kp, 0:Mo]),
                                rhs=mm(pooled[d][0:Kp, WH + 1 : WH + 3]), start=True, stop=True,
                            )
                        nc.scalar.copy(
                            out=osb[0:Mo, CH * jh : CH * (jh + 1)], in_=bps[0:Mo]
                        )
                        if corr == "L":
                            nc.vector.tensor_add(
                                out=osb[0:Mo, 0:1], in0=osb[0:Mo, 0:1],
                                in1=cps[0:Mo, 1:2],
                            )
                        elif corr == "R":
                            nc.vector.tensor_add(
                                out=osb[0:Mo, WH // 2 - 1 : WH // 2],
                                in0=osb[0:Mo, WH // 2 - 1 : WH // 2],
                                in1=cps[0:Mo, 0:1],
                            )
                    nc.sync.dma_start(
                        out=outdram[BM * d : BM * d + Mo,
                                    (WH // 2) * half : (WH // 2) * (half + 1)],
                        in_=osb[0:Mo],
                    )

    nc.compile()
    return nc


def get_program():
    if "nc" not in _PROGRAM:
        _PROGRAM["nc"] = _build_program()
    return _PROGRAM["nc"]


def build_in_maps(x2, w1, w2, w3, w4):
    in_maps = []
    for c in range(NCORES):
        m = {"x": _local_x(x2, c), "zpad": np.zeros((128, 4), np.float32)}
        m.update(_core_weights(c, w1, w2, w3, w4))
        in_maps.append(m)
    return in_maps


def kernel(x, w1, w2, w3, w4, H=None, W=None, nTh=None, nTw=None, **_):
    from concourse.bass_utils import run_bass_kernel_spmd

    x2 = np.asarray(x, dtype=np.float32).reshape(8192, 8192)
    ws = [np.asarray(w, dtype=np.float32).reshape(3, 3) for w in (w1, w2, w3, w4)]
    nc = get_program()
    in_maps = build_in_maps(x2, *ws)
    res = run_bass_kernel_spmd(nc, in_maps, core_ids=list(range(NCORES)))
    out = np.concatenate([res.results[c]["out"] for c in range(NCORES)], axis=0)
    return out.reshape(1, 1, 4096, 4096).astype(np.float32)
